# revision 27
# baseline (speedup 1.0000x reference)
"""Trainium2 Bass kernel for CardAwarePolicy (counts-reformulated MHA + folded MLPs).

Self-contained: takes full unsharded inputs, shards batch across 8 NeuronCores
(pure data parallel), runs a Tile/Bass kernel per core, gathers the output.

Math summary (per batch element, validated against the reference in numpy):
  The masked 4-head self-attention over the 8 hand slots depends on the hand
  only through its card-count vector n[c] (c in 0..53), because
  sum_q sum_k attn[h,q,k] v[k] collapses to card-vocabulary sums once the
  softmax exp() is folded into a precomputed table EG0[h,c',c] (stabilized,
  with the pad-card column zeroed).  With Nsc = n * (1/hand_len):
      den  = EG0 @ Nsc                  (per head, 54 query-cards)
      T    = Nrep / den                 (Nrep = Nsc stacked per head-pair)
      W2   = EG0^T @ T
      Y    = W2 * Nrep
      hand-term of ctx1 = BIG @ Y       (BIG folds V-table and out_w/ctx_w1)
  All other branches (enemy embed, game-state/discard MLPs, ctx MLP, action
  scorer) are folded into matmuls with the tiny weights pre-combined on host.

v3 engine plan (measured op costs on this silicon):
  - attention-side stays fp32/f32r: gpsimd (pool) is fast only on all-fp32
    SBUF tensor_tensor (~0.69us) and cannot touch PSUM.
  - PSUM->SBUF conversions (w2A/w2B copies, gd1r, ctx1s, u4s, out bias) on
    the scalar (ACT) engine, ~0.68us each, per-partition bias is free there.
  - DVE runs only the reciprocal (custom ISA) and the five scorer relus as
    tensor_scalar: bf16 SBUF TS ops run ~0.35us when clustered.
  - scorer tail in bf16 (ctx1s/u4s/H tiles + uc4/sc tables).
  - exu rows (enemy-emb, 1/len) ride in the X input tile (rows 108:121) and
    the Y(A) product overwrites X[0:108] in place, so bigA consumes X直接.
"""

import sys
import numpy as np
import ml_dtypes

sys.path.insert(0, "/opt/trn_rl_repo")

BF16 = ml_dtypes.bfloat16

B_FULL = 65536
N_CORES = 8
BC = B_FULL // N_CORES        # 8192 per core
TN = 512                      # batch columns per tile (= matmul free dim)
NT = BC // TN                 # 16 tiles per core
CHUNK = 4                     # tiles per xg/out DMA chunk
NH, HD, E, HS, A = 4, 3, 12, 8, 20

_CACHE = {}


# ---------------------------------------------------------------- host folding
def _fold_tables(inp):
    f = lambda k: np.asarray(inp[k], np.float64)
    card_emb, enemy_emb = f("card_emb"), f("enemy_emb")
    in_w, in_b = f("in_w"), f("in_b")
    out_w, out_b = f("out_w"), f("out_b")
    gs_w1, gs_b1, gs_w2, gs_b2 = f("gs_w1"), f("gs_b1"), f("gs_w2"), f("gs_b2")
    dp_w1, dp_b1, dp_w2, dp_b2 = f("dp_w1"), f("dp_b1"), f("dp_w2"), f("dp_b2")
    ctx_w1, ctx_b1, ctx_w2, ctx_b2 = f("ctx_w1"), f("ctx_b1"), f("ctx_w2"), f("ctx_b2")
    sc_w1, sc_b1, sc_w2, sc_b2 = f("sc_w1"), f("sc_b1"), f("sc_w2"), f("sc_b2")
    aci = np.asarray(inp["action_card_indices"])
    nva = int(inp["num_valid_actions"])

    Tq = card_emb @ in_w[0:12].T + in_b[0:12]
    Tk = card_emb @ in_w[12:24].T + in_b[12:24]
    Tv = card_emb @ in_w[24:36].T + in_b[24:36]
    G = np.zeros((NH, 54, 54))
    for h in range(NH):
        G[h] = (Tq[:, 3 * h:3 * h + 3] @ Tk[:, 3 * h:3 * h + 3].T) / np.sqrt(HD)
    EG0 = np.exp(G - G.max(axis=2, keepdims=True))
    EG0[:, :, 0] = 0.0

    T = {}

    def den_lhsT(heads):
        out = np.zeros((54, 108))
        for j, h in enumerate(heads):
            out[:, 54 * j:54 * j + 54] = EG0[h].T
        return out

    def w2_lhsT(heads):
        out = np.zeros((108, 108))
        for j, h in enumerate(heads):
            out[54 * j:54 * j + 54, 54 * j:54 * j + 54] = EG0[h]
        return out

    W1hh = ctx_w1[:, 0:12] @ out_w
    u0 = 8.0 * (ctx_w1[:, 0:12] @ out_b)

    def big_lhsT(heads):
        out = np.zeros((108, 128))
        for j, h in enumerate(heads):
            out[54 * j:54 * j + 54, :] = Tv[:, 3 * h:3 * h + 3] @ W1hh[:, 3 * h:3 * h + 3].T
        return out

    T["t_denA"], T["t_denB"] = den_lhsT((0, 1)), den_lhsT((2, 3))
    T["t_w2A"], T["t_w2B"] = w2_lhsT((0, 1)), w2_lhsT((2, 3))
    bigA = np.zeros((121, 128))
    bigA[0:108] = big_lhsT((0, 1))
    bigA[108:120, :] = ctx_w1[:, 12:24].T
    bigA[120, :] = u0
    T["t_bigA"] = bigA
    T["t_bigB"] = big_lhsT((2, 3))

    t_gd = np.zeros((66, 128))
    t_gd[0:12, 0:64] = gs_w1.T
    t_gd[12:66, 64:128] = dp_w1.T
    T["t_gd"] = t_gd
    b_gd = np.concatenate([gs_b1, dp_b1])[:, None]

    t_agd = np.zeros((128, 128))
    t_agd[0:64, :] = (ctx_w1[:, 24:30] @ gs_w2).T
    t_agd[64:128, :] = (ctx_w1[:, 30:36] @ dp_w2).T
    T["t_agd"] = t_agd

    bias_ctx1 = ctx_b1 + ctx_w1[:, 24:30] @ gs_b2 + ctx_w1[:, 30:36] @ dp_b2
    b_ctx1 = bias_ctx1[:, None]

    W_uc = sc_w1[:, 0:128] @ ctx_w2
    t_uc4 = np.zeros((128, 128))
    for a in range(4):
        t_uc4[:, 32 * a:32 * a + 32] = W_uc.T
    T["t_uc4"] = t_uc4

    am = (aci != 0).astype(np.float64)
    cnt = np.maximum(am.sum(axis=1), 1.0)
    arep = (card_emb[aci] * am[:, :, None]).sum(axis=1) / cnt[:, None]
    v = arep @ sc_w1[:, 128:140].T + sc_b1 + sc_w1[:, 0:128] @ ctx_b2  # [20,32]
    b_H = np.zeros((128, 5))
    for g in range(5):
        for a in range(4):
            b_H[32 * a:32 * a + 32, g] = v[4 * g + a]

    for g in range(5):
        t = np.zeros((128, 20))
        for a in range(4):
            t[32 * a:32 * a + 32, 4 * g + a] = sc_w2[0]
        T[f"t_sc{g}"] = t

    b_out = np.full((20, 1), float(np.asarray(sc_b2).reshape(-1)[0]))
    b_out[nva:] = -1e8

    # bias blob (fp32): columns
    #   0: b_gd, 1: b_ctx1, 2: bH0, 3: dbH1, 4: dbH2, 5: bH3, 6: bH4,
    #   7: b_out (20 rows)
    BB = np.zeros((128, 8))
    BB[:, 0] = b_gd[:, 0]
    BB[:, 1] = b_ctx1[:, 0]
    BB[:, 2] = b_H[:, 0]
    BB[:, 3] = b_H[:, 1] - b_H[:, 0]
    BB[:, 4] = b_H[:, 2] - b_H[:, 0]
    BB[:, 5] = b_H[:, 3]
    BB[:, 6] = b_H[:, 4]
    BB[0:20, 7] = b_out[:, 0]
    return T, np.ascontiguousarray(BB, np.float32)


# bf16 weight blob: den + gd tables first so the first DMA chunk unblocks
# tile-0 compute.
BLOB_LAYOUT = [  # name, rows, cols
    ("t_denA", 54, 108), ("t_denB", 54, 108), ("t_gd", 66, 128),
    ("t_w2A", 108, 108), ("t_w2B", 108, 108),
    ("t_bigA", 121, 128), ("t_bigB", 108, 128), ("t_agd", 128, 128),
    ("t_uc4", 128, 128),
    ("t_sc0", 128, 20), ("t_sc1", 128, 20), ("t_sc2", 128, 20),
    ("t_sc3", 128, 20), ("t_sc4", 128, 20),
]
BLOB_COLS = sum(c for _, _, c in BLOB_LAYOUT)
WB_SPLIT = 108 + 108 + 128   # den tables + gd table in the first DMA


def _pack_blobs(T):
    wb = np.zeros((128, BLOB_COLS), BF16)
    off = 0
    for name, rows, cols in BLOB_LAYOUT:
        wb[0:rows, off:off + cols] = T[name].astype(BF16)
        off += cols
    return wb


# ---------------------------------------------------------------- bass module
def _build_module(bc):
    import concourse.bass as bass
    import concourse.bacc as bacc
    import concourse.mybir as mybir
    from concourse import tile

    dt = mybir.dt
    f32, f32r, bf16 = dt.float32, dt.float32r, dt.bfloat16
    Alu = mybir.AluOpType
    Act = mybir.ActivationFunctionType
    nt = bc // TN

    nc = bacc.Bacc("TRN2", target_bir_lowering=False, debug=False)

    din = lambda name, shape, dtype: nc.dram_tensor(name, list(shape), dtype, kind="ExternalInput").ap()
    wb_d = din("wblob", (128, BLOB_COLS), bf16)
    bb_d = din("bblob", (128, 8), f32)
    x_d = din("xin", (nt, 121, TN), bf16)       # nsc(54) | nsc dup(54) | enemy(12) | rlen(1)
    xg_d = din("xg", (66, bc), bf16)            # game_state(12) | discard(54)
    out_d = nc.dram_tensor("out", [20, bc], f32, kind="ExternalOutput").ap()

    with tile.TileContext(nc) as tc:
        with (
            tc.tile_pool(name="const", bufs=1) as cpool,
            tc.tile_pool(name="io", bufs=4) as io,
            tc.tile_pool(name="work", bufs=2) as wk,
            tc.tile_pool(name="ps", bufs=1, space="PSUM") as ps,
        ):
            wblob = cpool.tile([128, BLOB_COLS], bf16, name="wblob")
            bblob = cpool.tile([128, 8], f32, name="bblob")
            # startup order: den/gd tables + biases first (sync), the rest on
            # the scalar queue so X(0)/xg(0) aren't stuck behind them.
            nc.sync.dma_start(out=wblob[:, 0:WB_SPLIT], in_=wb_d[:, 0:WB_SPLIT])
            nc.sync.dma_start(out=bblob, in_=bb_d)
            nc.scalar.dma_start(out=wblob[:, WB_SPLIT:], in_=wb_d[:, WB_SPLIT:])
            tb = {}
            off = 0
            for name, rows, cols in BLOB_LAYOUT:
                tb[name] = wblob[0:rows, off:off + cols]
                off += cols
            b_gd = bblob[:, 0:1]
            b_ctx1 = bblob[:, 1:2]
            bH0 = bblob[:, 2:3]
            dbH1 = bblob[:, 3:4]
            dbH2 = bblob[:, 4:5]
            bH3 = bblob[:, 5:6]
            bH4 = bblob[:, 6:7]
            b_out = bblob[0:20, 7:8]

            xg_tiles = {}
            out_tiles = {}
            for t in range(nt):
                c = t // CHUNK
                ccol = slice((t % CHUNK) * TN, (t % CHUNK + 1) * TN)

                # --- input DMAs (chunked xg/out, per-tile X) ---
                X = io.tile([121, TN], bf16, tag="x", bufs=6, name=f"x_{t}")
                nc.sync.dma_start(out=X, in_=x_d[t])
                if t % CHUNK == 0:
                    xg_tiles[c] = io.tile([66, CHUNK * TN], bf16, tag="xg", bufs=3,
                                          name=f"xg_{c}")
                    nc.sync.dma_start(out=xg_tiles[c],
                                      in_=xg_d[:, c * CHUNK * TN:(c + 1) * CHUNK * TN])
                    out_tiles[c] = wk.tile([20, CHUNK * TN], f32, tag="outc", bufs=2,
                                           name=f"outc_{c}")

                # --- hand branch (counts formulation) ---
                dAB_ps = ps.tile([108, 2 * TN], f32, tag="dAB", name=f"dAB_{t}")
                nc.tensor.matmul(dAB_ps[:, 0:TN], tb["t_denA"], X[0:54, :],
                                 start=True, stop=True)
                nc.tensor.matmul(dAB_ps[:, TN:2 * TN], tb["t_denB"], X[0:54, :],
                                 start=True, stop=True)
                rdAB = wk.tile([108, 2 * TN], f32, tag="rdAB", name=f"rdAB_{t}")
                nc.vector.reciprocal_approx_fast(out=rdAB[:, 0:TN],
                                                 in_=dAB_ps[:, 0:TN])
                nc.vector.reciprocal_approx_fast(out=rdAB[:, TN:2 * TN],
                                                 in_=dAB_ps[:, TN:2 * TN])

                TA = wk.tile([108, TN], bf16, tag="TA", name=f"TA_{t}")
                nc.gpsimd.tensor_tensor(TA, X[0:108, :], rdAB[:, 0:TN], Alu.mult)
                TB = wk.tile([108, TN], bf16, tag="TB", name=f"TB_{t}")
                nc.gpsimd.tensor_tensor(TB, X[0:108, :], rdAB[:, TN:2 * TN], Alu.mult)

                w2A_ps = ps.tile([108, TN], f32, tag="w2A", name=f"w2A_{t}")
                nc.tensor.matmul(w2A_ps, tb["t_w2A"], TA, start=True, stop=True)
                w2B_ps = ps.tile([108, TN], f32, tag="w2B", name=f"w2B_{t}")
                nc.tensor.matmul(w2B_ps, tb["t_w2B"], TB, start=True, stop=True)

                # Y products on DVE (PSUM-capable), YB first then in-place YA
                YB = wk.tile([108, TN], bf16, tag="YB", name=f"YB_{t}")
                nc.vector.tensor_tensor(YB, w2B_ps, X[0:108, :], Alu.mult)
                # in-place: X[0:108] <- w2A * X[0:108]; rows 108:121 stay (exu)
                nc.vector.tensor_tensor(X[0:108, :], w2A_ps, X[0:108, :], Alu.mult)

                # --- game-state / discard encoders ---
                gd1_ps = ps.tile([128, TN], f32, tag="gd1", name=f"gd1_{t}")
                nc.tensor.matmul(gd1_ps, tb["t_gd"], xg_tiles[c][:, ccol],
                                 start=True, stop=True)
                gd1r = wk.tile([128, TN], bf16, tag="gd1r", name=f"gd1r_{t}")
                nc.scalar.activation(gd1r, gd1_ps, Act.Relu, bias=b_gd, scale=1.0)

                # --- ctx layer 1 accumulation ---
                ctx1_ps = ps.tile([128, TN], f32, tag="ctx1", name=f"ctx1_{t}")
                nc.tensor.matmul(ctx1_ps, tb["t_bigA"], X, start=True, stop=False)
                nc.tensor.matmul(ctx1_ps, tb["t_bigB"], YB, start=False, stop=False)
                nc.tensor.matmul(ctx1_ps, tb["t_agd"], gd1r, start=False, stop=True)

                ctx1 = wk.tile([128, TN], bf16, tag="ctx1s", name=f"ctx1s_{t}")
                with tc.high_priority(offset=40):
                    nc.scalar.activation(ctx1, ctx1_ps, Act.Relu, bias=b_ctx1,
                                         scale=1.0)

                # --- scorer (bf16) ---
                u4_ps = ps.tile([128, TN], f32, tag="u4", name=f"u4_{t}")
                nc.tensor.matmul(u4_ps, tb["t_uc4"], ctx1, start=True, stop=True)

                # u4s = u4 + bH0 (bf16, ACT); H_g = relu(u4s + dbH_g) on DVE
                u4s = wk.tile([128, TN], bf16, tag="u4s", name=f"u4s_{t}")
                nc.scalar.activation(u4s, u4_ps, Act.Identity, bias=bH0, scale=1.0)
                Hs = []
                for g in range(5):
                    H = wk.tile([128, TN], bf16, tag=f"H{g}", name=f"H{g}_{t}")
                    Hs.append(H)
                nc.vector.tensor_scalar(Hs[0], u4s, 0.0, None, Alu.max)
                nc.vector.tensor_scalar(Hs[1], u4s, dbH1, 0.0, Alu.add, Alu.max)
                nc.vector.tensor_scalar(Hs[2], u4s, dbH2, 0.0, Alu.add, Alu.max)
                nc.scalar.activation(Hs[3], u4_ps, Act.Relu, bias=bH3, scale=1.0)
                nc.scalar.activation(Hs[4], u4_ps, Act.Relu, bias=bH4, scale=1.0)

                sc_ps = ps.tile([20, TN], f32, tag="sc", name=f"sc_{t}")
                for g, H in enumerate(Hs):
                    nc.tensor.matmul(sc_ps, tb[f"t_sc{g}"], H,
                                     start=(g == 0), stop=(g == 4))

                nc.scalar.activation(out_tiles[c][:, ccol], sc_ps, Act.Identity,
                                     bias=b_out, scale=1.0)
                if t % CHUNK == CHUNK - 1:
                    nc.gpsimd.dma_start(
                        out=out_d[:, c * CHUNK * TN:(c + 1) * CHUNK * TN],
                        in_=out_tiles[c])

    nc.finalize()
    return nc


def _get_module(bc=BC):
    key = ("mod", bc)
    if key not in _CACHE:
        _CACHE[key] = _build_module(bc)
    return _CACHE[key]


# ---------------------------------------------------------------- host prep
def _prep_data(inp):
    """Full-batch host prep: counts, scaling, layout. Returns per-core input maps."""
    hc = np.asarray(inp["hand_cards"])
    B = hc.shape[0]
    gs = np.asarray(inp["game_state"], np.float32)
    dp = np.asarray(inp["discard_pile_cards"], np.float32)
    en = np.asarray(inp["enemy_card"]).reshape(B).astype(np.int64)
    hsz = np.asarray(inp["hand_size"]).astype(np.float64)

    idx = (hc.astype(np.int64) + 54 * np.arange(B, dtype=np.int64)[:, None]).ravel()
    counts = np.bincount(idx, minlength=B * 54).reshape(B, 54)
    rlen = (1.0 / np.maximum(hsz, 1.0)).astype(np.float32)
    nsc = (counts.astype(np.float32) * rlen[:, None]).T  # [54, B]

    en_emb = np.asarray(inp["enemy_emb"], np.float32)
    xall = np.empty((121, B), np.float32)
    xall[0:54] = nsc
    xall[54:108] = nsc
    xall[108:120] = en_emb[en].T
    xall[120] = rlen
    xall = xall.astype(BF16)

    xg = np.empty((66, B), np.float32)
    xg[0:12] = gs.T
    xg[12:66] = dp.T
    xg = xg.astype(BF16)

    tables, bb = _fold_tables(inp)
    wb = _pack_blobs(tables)

    maps = []
    for c in range(N_CORES):
        cols = slice(c * BC, (c + 1) * BC)
        x_c = np.ascontiguousarray(xall[:, cols])               # [121, BC]
        x_p = np.ascontiguousarray(
            x_c.reshape(121, NT, TN).transpose(1, 0, 2))        # [NT,121,TN]
        m = {"wblob": wb, "bblob": bb,
             "xin": x_p,
             "xg": np.ascontiguousarray(xg[:, cols])}
        maps.append(m)
    return maps


# ---------------------------------------------------------------- entry points
def _run(inputs, trace=False):
    from concourse.bass_utils import run_bass_kernel_spmd

    in_maps = _prep_data(inputs)
    nc = _get_module()
    res = run_bass_kernel_spmd(nc, in_maps, list(range(N_CORES)), trace=trace)
    out = np.concatenate([r["out"] for r in res.results], axis=1).T  # [B, 20]
    return np.ascontiguousarray(out), res


def kernel(**inputs) -> np.ndarray:
    out, _ = _run(inputs, trace=False)
    return out
